# revision 16
# baseline (speedup 1.0000x reference)
"""Trainium2 Bass kernel for nn_Actor_IntentionEncoder (gnn_message_passing).

Data-parallel over N = B*A = 8192 rows; core c owns rows [1024c, 1024c+1024)
= output groups [64c, 64c+64).

Phase 1 (hypernet) runs in transposed layout: partitions p = (d, s) with
d in [0,64), s in {0,1}; free axis = local row n. For each h'-pair
j in [0,64):
    wps[p, n] = sum_k hw2B[k, 128j+p] * hT[k, n]          (PE, 2x[128,512])
    ACT path: t = relu(wps + b2B[:, j])  (per-partition bias)
              m = t * xx                 (DVE tensor_tensor f16)
    DVE path: m = max(wps, -b2B[:, j]) * xx   (fused STT; relu(w+b) =
              max(w,-b)+b, the +x*b term is one K=64 corr matmul)
    emb_pre^T[2j+s, n] += sum_d m[d+64s, n]  (PE selector matmul, sliding
              window slice of a [128,126] 0/1 matrix, PSUM accumulate)
emb^T = tanh(emb_pre^T) (ACT). Group sums via DVE strided reduce ->
AllGather (8KB per half; half 0 fires mid-loop, half 1 right after) ->
phase 2 (value MLP + attention). mean_rep row n = mean of group n%512
(reference tiles, not repeat-interleaves), so the AllGather is required.

vs the first working version (132us): constants packed into 2 DRAM
tensors (2 DMA issues instead of ~21), input DMAs ordered by first use,
PE warm-up burst before the inputs land (HAM at 2.4GHz when real
matmuls start), selector lag 4 with deeper m-tile pool (j-loop 1093 ->
~1000 ns/j), a tiny "skew sponge" AllGather issued at kernel start (the
CC stream pays a ~30-50us cross-core barrier + launch skew on its first
op; the sponge absorbs that concurrently with the j-loop so the real
gathers run at fabric latency), attention y1a matmuls (bank-wide, one
PSUM start per bank -- start=True clears has_written for the WHOLE
bank) plus ~7us of throwaway keep-warm matmuls run during the
collective wait, ACT evacuations split in halves to pipeline with
consumers, P_w scaling on DVE instead of GPSIMD. ~121us measured;
remaining time is j-loop (PE-saturated at ~250ns per N=512 matmul,
64 j x 4 MMs) + the semantically-required AllGather latency + fixed
preamble/barrier costs.
"""

import sys

sys.path.insert(0, "/opt/trn_rl_repo")

import numpy as np

import concourse.bacc as bacc
import concourse.tile as tile
from concourse import mybir
from concourse.bass_utils import run_bass_kernel_spmd

F32 = mybir.dt.float32
F16 = mybir.dt.float16
AF = mybir.ActivationFunctionType
OP = mybir.AluOpType
NPF16 = np.float16

NCORES = 8
B, A, DS, DO, H = 512, 16, 32, 32, 128
D = DS + DO  # 64
N = B * A  # 8192
RPC = N // NCORES  # rows per core = 1024
GPC = B // NCORES  # groups per core = 64
NJ = 64  # h'-pairs
SEL_LAG = 4
WARMUP_MM = 56

# j-pairs whose evacuation goes through ACT (relu + per-partition bias);
# the rest go through the DVE fused max-mult path.
ACT_J = [j for j in range(NJ) if (j % 16) < 11]
ACT_JSET = set(ACT_J)

# f16 constant pack column layout (128 partitions x PACK16_COLS)
P16 = {
    "hw1": (0, 64, 0, 128),        # rows 0:64, cols 0:128
    "vb2row": (64, 65, 0, 128),
    "onescol": (64, 65, 254, 382),
    "selpad": (0, 128, 128, 254),
    "bcorr": (0, 64, 254, 382),
    "vw1": (0, 128, 382, 510),
    "vw2": (0, 128, 510, 638),
    "aw1a": (0, 128, 638, 766),
    "aw1b": (0, 128, 766, 894),
    "aw2": (0, 128, 894, 1022),
    "ones128": (0, 128, 1022, 1023),
    "aw3": (0, 128, 1023, 1024),
    "pmask": (0, 128, 1024, 1032),
}
PACK16_COLS = 1032

# f32 constant pack column layout
P32 = {
    "b2B": (0, 128, 0, 64),
    "negb2B": (0, 128, 64, 128),
    "hb1": (0, 128, 128, 129),
    "vb1": (0, 128, 129, 130),
    "ab1": (0, 128, 130, 131),
    "ab2": (0, 128, 131, 132),
    "ab3": (0, 128, 132, 133),
}
PACK32_COLS = 133

_CACHE = {}


def _build():
    nc = bacc.Bacc("TRN2", target_bir_lowering=False, debug=False,
                   num_devices=NCORES)

    def inp(name, shape, dt=F32):
        return nc.dram_tensor(name, list(shape), dt, kind="ExternalInput").ap()

    xxT_d = inp("xxT", (128, RPC), F16)
    pack32_d = inp("pack32", (128, PACK32_COLS))
    pack16_d = inp("pack16", (128, PACK16_COLS), F16)
    hw2B_d = inp("hw2B", (H, D * H), F16)

    out_d = nc.dram_tensor("out", [GPC, H], F32, kind="ExternalOutput").ap()

    with tile.TileContext(nc) as tc:
        with (
            tc.tile_pool(name="const", bufs=1) as constp,
            tc.tile_pool(name="keep", bufs=1) as keepp,
            tc.tile_pool(name="work", bufs=6) as workp,
            tc.tile_pool(name="wps", bufs=3, space="PSUM") as wpsp,
            tc.tile_pool(name="embps", bufs=1, space="PSUM") as embpsp,
            tc.tile_pool(name="dram", bufs=1, space="DRAM") as dramp,
        ):
            # ---- PE warm-up: dense tiny matmuls on a zeroed scratch tile,
            # issued before any DMA lands so HAM reaches 8/8 by the time
            # the real pipeline starts. ~3.5us, hidden under the input DMA.
            warm_sb = constp.tile([64, 64], F16, tag="warm", name="warm_sb")
            nc.gpsimd.memset(warm_sb[:], 0.0)
            warmps = wpsp.tile([128, RPC], F32, tag="wps", name="warmps")
            for _ in range(WARMUP_MM):
                nc.tensor.matmul(warmps[0:64, 0:64], warm_sb[:], warm_sb[:],
                                 start=True, stop=True)

            # ---- loads, ordered by first use: xxT half 0 (hT h0), biases,
            # f16 consts, hw2B chunk 0 (hyper j=0), xxT half 1, rest of hw2B.
            xxT_sb = constp.tile([128, RPC], F16, tag="xxT", name="xxT_sb")
            pk32 = constp.tile([128, PACK32_COLS], F32, tag="pack32",
                               name="pack32_sb")
            pk16 = constp.tile([128, PACK16_COLS], F16, tag="pack16",
                               name="pack16_sb")
            hw2B_sb = constp.tile([H, D * H], F16, tag="hw2B")
            nc.sync.dma_start(xxT_sb[:, 0:512], xxT_d[:, 0:512])
            nc.sync.dma_start(pk16[:, 0:382], pack16_d[:, 0:382])
            nc.sync.dma_start(pk32[:], pack32_d[:])
            nc.sync.dma_start(hw2B_sb[:, 0:1024], hw2B_d[:, 0:1024])
            nc.sync.dma_start(xxT_sb[:, 512:1024], xxT_d[:, 512:1024])
            nc.sync.dma_start(pk16[:, 382:PACK16_COLS],
                              pack16_d[:, 382:PACK16_COLS])
            for s in range(1, 8):
                nc.sync.dma_start(hw2B_sb[:, s * 1024:(s + 1) * 1024],
                                  hw2B_d[:, s * 1024:(s + 1) * 1024])

            # skew sponge: a tiny AllGather issued early. The CC stream pays
            # a ~30-50us cross-core barrier + launch skew on its first op;
            # this absorbs both concurrently with the j-loop so the real
            # gathers run at fabric latency.
            spz = workp.tile([1, 16], F16, tag="spz", name="spz")
            nc.gpsimd.memset(spz[:], 0.0)
            sp_in = dramp.tile([1, 16], F16, tag="sp_in", name="sp_in")
            sp_out = dramp.tile([NCORES, 16], F16, tag="sp_out",
                                name="sp_out")
            nc.sync.dma_start(sp_in[:], spz[:])
            nc.gpsimd.collective_compute(
                "AllGather", OP.bypass,
                replica_groups=[list(range(NCORES))],
                ins=[sp_in.opt()], outs=[sp_out.opt()])

            def c16(key):
                r0, r1, c0, c1 = P16[key]
                return pk16[r0:r1, c0:c1]

            def c32(key):
                r0, r1, c0, c1 = P32[key]
                return pk32[r0:r1, c0:c1]

            hw1_sb = c16("hw1")
            selpad_sb = c16("selpad")
            bcorr_sb = c16("bcorr")
            vw1_sb, vw2_sb = c16("vw1"), c16("vw2")
            vb2r_sb, onescol_sb = c16("vb2row"), c16("onescol")
            aw1a_sb, aw1b_sb, aw2_sb = c16("aw1a"), c16("aw1b"), c16("aw2")
            ones128_sb, aw3_sb, pmask_sb = c16("ones128"), c16("aw3"), c16("pmask")
            b2B_sb, negb2B_sb = c32("b2B"), c32("negb2B")
            hb1_sb, vb1_sb = c32("hb1"), c32("vb1")
            ab1_sb, ab2_sb, ab3_sb = c32("ab1"), c32("ab2"), c32("ab3")

            # ---- hT = relu(hw1^T @ x^T + hb1)  [128, 1024] f16 ----
            hps = wpsp.tile([128, RPC], F32, tag="wps", name="hps")
            for h in range(2):
                nc.tensor.matmul(hps[:, h * 512:(h + 1) * 512], hw1_sb[:],
                                 xxT_sb[0:D, h * 512:(h + 1) * 512],
                                 start=True, stop=True)
            hT_sb = keepp.tile([H, RPC], F16, tag="hT")
            for h in range(2):
                nc.scalar.activation(hT_sb[:, h * 512:(h + 1) * 512],
                                     hps[:, h * 512:(h + 1) * 512],
                                     AF.Relu, bias=hb1_sb[:])

            # ---- phase 1: hypernet j-loop ----
            embps = embpsp.tile([128, RPC], F32, tag="embps")
            for h in range(2):
                nc.tensor.matmul(embps[:, h * 512:(h + 1) * 512], bcorr_sb[:],
                                 xxT_sb[0:D, h * 512:(h + 1) * 512],
                                 start=True, stop=True)

            m_tiles = [None] * NJ

            def emit_hyper(j):
                wps = wpsp.tile([128, RPC], F32, tag="wps", name=f"wps_{j}")
                for h in range(2):
                    nc.tensor.matmul(wps[:, h * 512:(h + 1) * 512],
                                     hw2B_sb[:, j * 128:(j + 1) * 128],
                                     hT_sb[:, h * 512:(h + 1) * 512],
                                     start=True, stop=True)
                m = workp.tile([128, RPC], F16, tag="m", name=f"m_{j}")
                if j in ACT_JSET:
                    t = workp.tile([128, RPC], F16, tag="t", name=f"t_{j}")
                    nc.scalar.activation(t[:], wps[:], AF.Relu,
                                         bias=b2B_sb[:, j:j + 1])
                    nc.vector.tensor_tensor(m[:], t[:], xxT_sb[:], op=OP.mult)
                else:
                    nc.vector.scalar_tensor_tensor(
                        m[:], wps[:], negb2B_sb[:, j:j + 1], xxT_sb[:],
                        OP.max, OP.mult)
                m_tiles[j] = m

            def emit_selector(j):
                m = m_tiles[j]
                q = 64 * (j // 32)
                c0 = 62 - 2 * (j % 32)
                for h in range(2):
                    nc.tensor.matmul(
                        embps[q:q + 64, h * 512:(h + 1) * 512],
                        selpad_sb[:, c0:c0 + 64],
                        m[:, h * 512:(h + 1) * 512],
                        start=False, stop=True, skip_group_check=True)
                m_tiles[j] = None

            embT_sb = keepp.tile([128, RPC], F16, tag="embT")
            cc_in, cc_out = [], []
            for half in range(2):
                cc_in.append(dramp.tile([64, GPC], F16, tag=f"cc_in{half}",
                                        name=f"cc_in{half}"))
                cc_out.append(dramp.tile([NCORES * 64, GPC], F16,
                                         tag=f"cc_out{half}",
                                         name=f"cc_out{half}"))

            def emit_gather(half):
                rows = slice(64 * half, 64 * half + 64)
                nc.scalar.activation(embT_sb[rows, :], embps[rows, :], AF.Tanh)
                msumh = workp.tile([64, GPC], F16, tag="msumh",
                                   name=f"msumh{half}")
                with nc.allow_low_precision(reason="group sums go to f16 "
                                            "for the AllGather anyway"):
                    nc.vector.tensor_reduce(
                        msumh[:],
                        embT_sb[rows, :].rearrange("p (g a) -> p g a", a=A),
                        axis=mybir.AxisListType.X, op=OP.add)
                nc.sync.dma_start(cc_in[half][:], msumh[:])
                nc.gpsimd.collective_compute(
                    "AllGather", OP.bypass,
                    replica_groups=[list(range(NCORES))],
                    ins=[cc_in[half].opt()], outs=[cc_out[half].opt()])

            for j in range(NJ + SEL_LAG):
                if j < NJ:
                    emit_hyper(j)
                if j >= SEL_LAG:
                    emit_selector(j - SEL_LAG)
                if j - SEL_LAG == 31:
                    emit_gather(0)
            emit_gather(1)

            # ---- phase 2a (overlaps collective): value MLP + y1a ----
            def chsl(t, ch):
                return t[:, ch * 128:(ch + 1) * 128]

            v1ps = wpsp.tile([128, RPC], F32, tag="wps", name="v1ps")
            for ch in range(8):
                nc.tensor.matmul(chsl(v1ps, ch), vw1_sb[:], chsl(embT_sb, ch),
                                 start=True, stop=True)
            v1T = workp.tile([128, RPC], F16, tag="m", name="v1T")
            for h in range(2):
                nc.scalar.activation(v1T[:, h * 512:(h + 1) * 512],
                                     v1ps[:, h * 512:(h + 1) * 512],
                                     AF.Relu, bias=vb1_sb[:])

            vps = wpsp.tile([128, RPC], F32, tag="wps", name="vps")
            for ch in range(8):
                nc.tensor.matmul(chsl(vps, ch), onescol_sb[:], vb2r_sb[:],
                                 start=True, stop=False)
                nc.tensor.matmul(chsl(vps, ch), chsl(v1T, ch), vw2_sb[:],
                                 start=False, stop=True)
            vals_sb = keepp.tile([128, RPC], F16, tag="vals")
            for h in range(2):
                nc.scalar.activation(vals_sb[:, h * 512:(h + 1) * 512],
                                     vps[:, h * 512:(h + 1) * 512], AF.Relu)

            # y1a does not need the collective result: run it under the CC
            y1ps = embpsp.tile([128, RPC], F32, tag="embps", name="y1ps")
            for h in range(2):
                nc.tensor.matmul(y1ps[:, h * 512:(h + 1) * 512], aw1a_sb[:],
                                 embT_sb[:, h * 512:(h + 1) * 512],
                                 start=True, stop=True)

            # keep the PE HAM-warm across the collective wait: ~6us of
            # throwaway matmuls on data that is already resident. They run
            # in the PE-idle window (CC fabric latency ~8.5us) and drain
            # before the gathered means arrive.
            dummyps = wpsp.tile([128, RPC], F32, tag="wps", name="dummyps")
            for _ in range(24):
                nc.tensor.matmul(dummyps[0:64, 0:512], warm_sb[:],
                                 hT_sb[0:64, 0:512], start=True, stop=True)

            # P_w tiles zeroed on gpsimd during the collective
            P_w = []
            for ch in range(8):
                pw = workp.tile([128, GPC], F16, tag=f"P_w{ch}",
                                name=f"P_w_{ch}")
                nc.gpsimd.memset(pw[:], 0.0)
                P_w.append(pw)

            meanT_full = keepp.tile([128, B], F16, tag="meanTf")
            for half in range(2):
                rows = slice(64 * half, 64 * half + 64)
                nc.sync.dma_start(
                    meanT_full[rows, :].rearrange("p (c g) -> p c g",
                                                  c=NCORES),
                    cc_out[half][:].rearrange("(c p) g -> p c g", c=NCORES))

            # ---- phase 2b: attention + weighted output ----
            for ch in range(8):
                mcol = 128 * (ch % 4)
                nc.tensor.matmul(chsl(y1ps, ch), aw1b_sb[:],
                                 meanT_full[:, mcol:mcol + 128],
                                 start=False, stop=True,
                                 skip_group_check=True)
            y1T = workp.tile([128, RPC], F16, tag="m", name="y1T")
            for h in range(2):
                nc.scalar.activation(y1T[:, h * 512:(h + 1) * 512],
                                     y1ps[:, h * 512:(h + 1) * 512],
                                     AF.Relu, bias=ab1_sb[:])

            y2ps = wpsp.tile([128, RPC], F32, tag="wps", name="y2ps")
            for ch in range(8):
                nc.tensor.matmul(chsl(y2ps, ch), aw2_sb[:], chsl(y1T, ch),
                                 start=True, stop=True)
            y2T = workp.tile([128, RPC], F16, tag="m", name="y2T")
            for h in range(2):
                nc.scalar.activation(y2T[:, h * 512:(h + 1) * 512],
                                     y2ps[:, h * 512:(h + 1) * 512],
                                     AF.Relu, bias=ab2_sb[:])

            scps_t = wpsp.tile([128, RPC], F32, tag="wps", name="scps")
            for ch in range(8):
                nc.tensor.matmul(scps_t[:, ch:ch + 1], chsl(y2T, ch),
                                 aw3_sb[:], start=True, stop=True)
            exp_sb = workp.tile([128, 8], F32, tag="exp_sb")
            nc.scalar.activation(exp_sb[:], scps_t[:, 0:8], AF.Exp,
                                 bias=ab3_sb[:])

            so_t = wpsp.tile([128, RPC], F32, tag="wps", name="so")
            for ch in range(8):
                nc.vector.tensor_scalar_mul(P_w[ch][:, ch * 8:(ch + 1) * 8],
                                            pmask_sb[:], exp_sb[:, ch:ch + 1])
                nc.tensor.matmul(so_t[0:GPC, 0:128], P_w[ch][:],
                                 chsl(vals_sb, ch),
                                 start=(ch == 0), stop=(ch == 7))
            for ch in range(8):
                nc.tensor.matmul(so_t[0:GPC, 128:129], P_w[ch][:],
                                 ones128_sb[:], start=(ch == 0), stop=(ch == 7))

            inv_S = workp.tile([GPC, 1], F32, tag="inv_S")
            nc.vector.reciprocal(inv_S[:], so_t[0:GPC, 128:129])
            out_sb = workp.tile([GPC, H], F32, tag="out_sb")
            nc.vector.tensor_scalar_mul(out_sb[:], so_t[0:GPC, 0:128],
                                        inv_S[:])
            nc.sync.dma_start(out_d[:], out_sb[:])

    nc.compile()
    return nc


def _prep_inputs(obs, latent, hw1, hb1, hw2, hb2, vw1, vb1, vw2, vb2,
                 aw1, ab1, aw2, ab2, aw3, ab3):
    f = np.float32
    fh = lambda a: np.asarray(a, f).astype(NPF16)

    x_full = np.concatenate(
        [np.tile(obs, (A, 1)), latent.reshape(-1, DO)], axis=1).astype(f)

    # hw2B[k, 128j + 64s + d] = hw2[k, 128d + 2j + s]
    hw2B = np.asarray(hw2, f).reshape(H, D, NJ, 2).transpose(0, 2, 3, 1) \
        .reshape(H, D * H)
    # b2B[64s + d, j] = hb2[128d + 2j + s]
    b2B = np.asarray(hb2, f).reshape(D, NJ, 2).transpose(2, 0, 1) \
        .reshape(128, NJ)
    # corr term only for DVE (max-trick) j's
    bcorr = np.asarray(hb2, f).reshape(D, H).copy()
    for j in ACT_J:
        bcorr[:, 2 * j] = 0.0
        bcorr[:, 2 * j + 1] = 0.0
    selpad = np.zeros((128, 126), NPF16)
    for p in range(128):
        selpad[p, 62 + p // 64] = 1.0
    pmask = np.zeros((128, 8), NPF16)
    for r in range(128):
        pmask[r, r // 16] = 1.0

    pk16 = np.zeros((128, PACK16_COLS), NPF16)

    def put16(key, arr):
        r0, r1, c0, c1 = P16[key]
        pk16[r0:r1, c0:c1] = fh(arr).reshape(r1 - r0, c1 - c0)

    put16("hw1", hw1)
    put16("vb2row", np.asarray(vb2).reshape(1, H))
    put16("onescol", np.ones((1, H)))
    put16("selpad", selpad)
    put16("bcorr", bcorr)
    put16("vw1", vw1)
    put16("vw2", vw2)
    put16("aw1a", np.asarray(aw1)[:H])
    put16("aw1b", np.asarray(aw1)[H:] / A)
    put16("aw2", aw2)
    put16("ones128", np.ones((H, 1)))
    put16("aw3", np.asarray(aw3).reshape(H, 1))
    put16("pmask", pmask)

    pk32 = np.zeros((128, PACK32_COLS), f)

    def put32(key, arr):
        r0, r1, c0, c1 = P32[key]
        pk32[r0:r1, c0:c1] = np.asarray(arr, f).reshape(r1 - r0, c1 - c0)

    put32("b2B", b2B)
    put32("negb2B", -b2B)
    put32("hb1", np.asarray(hb1).reshape(H, 1))
    put32("vb1", np.asarray(vb1).reshape(H, 1))
    put32("ab1", np.asarray(ab1).reshape(H, 1))
    put32("ab2", np.asarray(ab2).reshape(H, 1))
    put32("ab3", np.full((128, 1), np.float32(np.asarray(ab3).reshape(()))))

    shared = dict(
        pack16=np.ascontiguousarray(pk16),
        pack32=np.ascontiguousarray(pk32),
        hw2B=np.ascontiguousarray(fh(hw2B)),
    )
    in_maps = []
    for c in range(NCORES):
        xcT = x_full[c * RPC:(c + 1) * RPC].T  # [64, 1024]
        m = dict(shared)
        m["xxT"] = np.ascontiguousarray(
            np.concatenate([xcT, xcT], axis=0)).astype(NPF16)
        in_maps.append(m)
    return in_maps


def kernel(**inputs):
    obs = np.asarray(inputs["obs"], np.float32)
    latent = np.asarray(inputs["obs_intention_latent"], np.float32)
    in_maps = _prep_inputs(
        obs, latent, inputs["hw1"], inputs["hb1"], inputs["hw2"], inputs["hb2"],
        inputs["vw1"], inputs["vb1"], inputs["vw2"], inputs["vb2"],
        inputs["aw1"], inputs["ab1"], inputs["aw2"], inputs["ab2"],
        inputs["aw3"], inputs["ab3"])
    if "nc" not in _CACHE:
        _CACHE["nc"] = _build()
    res = run_bass_kernel_spmd(_CACHE["nc"], in_maps, list(range(NCORES)))
    _CACHE["res"] = res
    out = np.empty((B, H), np.float32)
    for c in range(NCORES):
        out[c * GPC:(c + 1) * GPC] = res.results[c]["out"]
    return out


if __name__ == "__main__":
    import reference
    inputs = reference.setup_inputs()
    inputs = {k: np.asarray(v) for k, v in inputs.items()}
    got = kernel(**inputs)
    exp = np.asarray(reference.reference(**reference.setup_inputs()))
    print("Relative error:", np.abs(got - exp).max() / (np.abs(exp).max() + 1e-9))
